# revision 18
# baseline (speedup 1.0000x reference)
"""Inverse STFT (nn_InverseSTFT) as a Bass/Tile kernel on 8 TRN2 NeuronCores.

Math
----
Reference computes, per batch b:
  full spectrum from one-sided stft via conjugate symmetry (F = 1024),
  ytmp[w, t] = sum_{f,c} full[f, t, c] * basis[f, w, c]          (IDFT)
  y = overlap_add(ytmp, hop=256), window-sum normalize, trim n_fft//2.

Folding the conjugate symmetry into the basis gives an exact K=1024 real
matmul; every folded-basis row k is a single-frequency sinusoid
g_k[w] = gamma * cos/sin(2*pi*f_k*w/1024). With hop = 1024/4, writing
w = 256*j + r factors each row as
  g_k[256 j + r] = cos(pi f_k j / 2) * g_k[r] + sin(pi f_k j / 2) * h_k[r]
with coefficients in {-1, 0, 1} determined by f_k mod 4 (h_k is the
quadrature partner of g_k). The overlap-add over j therefore collapses
into a shifted-add prefilter on the frames (computed on HOST, since it is
a cheap linear repack of the input) followed by matmuls of only
  K=1024 (U part)  +  K=512 (H part; only odd f has sin coefficients)
per 256-wide output segment, instead of 4 * K=1024:
  u[k, s] = sum_j c_{kj} x[k, s-j]   (c patterns by f mod 4:
            [1,1,1,1] / [1,0,-1,0] / [1,-1,1,-1] / [1,0,-1,0])
  v[k, s] = sum_j s_{kj} x[k, s-j] = u[k, s-1]  for odd f (free shift!)
  y[256 s + r] = sum_k Ub[k, r] u[k, s] + sum_{odd f} Hb[k, r] u[k, s-1]
K rows are permuted so each 128-chunk holds a single f-mod-4 class
(classes have exactly 256 rows each); the sign for f==3 mod 4's v is
folded into Hb. This is 12 accumulating chunk-matmuls per psum tile
instead of 32 -> 2.67x fewer TensorE cycles.

Schedule: chunk-outer / s-tile-inner with all 16 psum tiles (= all 8
PSUM banks) live per batch, so matmuls start as soon as u-chunk 0 lands
instead of waiting for the whole batch's DMA.

Window-sum normalization = 0.25 folded into the bases; per-partition
fixup on the two edge s-tiles. Output keeps segments s = 2..2002.

Sharding: pure data parallel, 2 batches per core.
"""

import numpy as np

import concourse.bass as bass
import concourse.mybir as mybir
from concourse.tile import TileContext
from concourse import bacc, bass_utils

N_FFT = 1024
HOP = 256
B = 16
NFREQ = 513
T = 2000
NCORES = 8
NB = B // NCORES          # batches per core
KC = 8                    # K chunks of 128 (K = 1024)
SU = 2052                 # u free size: i in [0, 2052), i <-> s = i - 1
SEG = 2003                # total segments in un-trimmed output
OUT_SEGS = 2001           # segments s = 2..2002
NT = 16                   # s-tiles of 128 per batch (last has 81 valid rows)
OUT_LEN = OUT_SEGS * HOP  # 512256
HCHUNKS = (2, 3, 6, 7)    # u chunks (f mod 4 == 1 or 3) used by the H part

F32 = mybir.dt.float32
DT_IN = mybir.dt.bfloat16

import ml_dtypes

NP_IN = ml_dtypes.bfloat16


def _make_bases():
    """(8,128,256) U basis and (4,128,256) H basis, 0.25 wss folded in.

    Row k of the folded basis (k<=512: cos rows f=k; k>512: sin rows
    f=k-512) restricted to r in [0,256), plus its quadrature partner.
    Rows permuted so chunks 0-1 = f%4==0, 2-3 = f%4==1, 4-5 = f%4==2,
    6-7 = f%4==3; the f%4==3 sin-coefficient sign is folded into Hb.
    """
    fk = np.concatenate([np.arange(513), np.arange(1, 512)])
    is_sin = np.concatenate([np.zeros(513, bool), np.ones(511, bool)])
    k = np.arange(1024)
    gamma = np.where((k == 0) | (k == 512), 1.0 / 1024, 2.0 / 1024)
    gamma = np.where(is_sin, -2.0 / 1024, gamma)
    r = np.arange(256)
    th = 2 * np.pi * np.outer(fk, r) / 1024.0
    g = np.where(is_sin[:, None], gamma[:, None] * np.sin(th),
                 gamma[:, None] * np.cos(th))
    h = np.where(is_sin[:, None], gamma[:, None] * np.cos(th),
                 -gamma[:, None] * np.sin(th))
    cls = fk % 4
    perm = np.concatenate([np.where(cls == c)[0] for c in range(4)])
    gp, hp, clsp = g[perm], h[perm], cls[perm]
    Ub = (gp * 0.25).reshape(KC, 128, 256)
    hrows = np.concatenate([np.where(clsp == 1)[0], np.where(clsp == 3)[0]])
    sign = np.where(clsp[hrows] == 1, 1.0, -1.0)[:, None]
    Hb = (hp[hrows] * sign * 0.25).reshape(4, 128, 256)
    return perm, Ub.astype(NP_IN), Hb.astype(NP_IN)


def _make_scales() -> np.ndarray:
    """(128, 2) per-partition wss fixup (on top of the 0.25 in the bases).

    col 0 -> first s-tile (s = 2..129): s=2 has 3 frames -> 4/3.
    col 1 -> last s-tile (s = 1922..2002): s=2000 -> 4/3, 2001 -> 2, 2002 -> 4.
    """
    sc = np.ones((128, 2), np.float32)
    sc[0, 0] = np.float32(4.0) / np.float32(3.0)
    sc[78, 1] = np.float32(4.0) / np.float32(3.0)
    sc[79, 1] = 2.0
    sc[80, 1] = 4.0
    return sc


def _prep_u(stft: np.ndarray, perm: np.ndarray) -> np.ndarray:
    """(16,513,2000,2) f32 -> (16, KC, 128, SU) prefiltered u, bf16.

    u[k, i] <-> u[k, s = i-1] = sum_j c_{kj} x[k, s-j], x zero outside
    [0, T). Computed in f32, cast to bf16 at the end.
    """
    re = stft[:, :, :, 0]                  # (B, 513, T)
    im = stft[:, 1:512, :, 1]              # (B, 511, T)
    xk = np.concatenate([re, im], axis=1)  # (B, 1024, T)
    xp = np.zeros((B, 1024, 2056), np.float32)
    xp[:, :, 4 : 4 + T] = xk[:, perm, :]   # xp[:, :, t+4] = x[t]
    u = np.empty((B, 1024, SU), np.float32)
    x0 = xp[:, :, 3 : 3 + SU]              # x[s]
    x1 = xp[:, :, 2 : 2 + SU]              # x[s-1]
    x2 = xp[:, :, 1 : 1 + SU]              # x[s-2]
    x3 = xp[:, :, 0 : SU]                  # x[s-3]
    u[:, 0:256] = x0[:, 0:256] + x1[:, 0:256] + x2[:, 0:256] + x3[:, 0:256]
    u[:, 512:768] = (x0[:, 512:768] - x1[:, 512:768]
                     + x2[:, 512:768] - x3[:, 512:768])
    u[:, 256:512] = x0[:, 256:512] - x2[:, 256:512]
    u[:, 768:1024] = x0[:, 768:1024] - x2[:, 768:1024]
    return u.reshape(B, KC, 128, SU).astype(NP_IN)


ACHUNKS = (0, 1, 4, 5)    # u chunks whose batch-0 tile carries u + Ub
SUA = SU + 256
SUB = SU + 512


def _fuse_inputs(U: np.ndarray, Ub: np.ndarray, Hb: np.ndarray):
    """Per-core input arrays: batch-0 chunk tiles carry their own basis
    columns (Ub, and Hb for the odd-class chunks) so one DMA per chunk
    delivers everything that chunk's matmul sweeps need."""
    ncores_in = U.shape[0] // NB
    u0a = np.zeros((ncores_in, 4, 128, SUA), NP_IN)
    u0b = np.zeros((ncores_in, 4, 128, SUB), NP_IN)
    for i, kc in enumerate(ACHUNKS):
        u0a[:, i, :, :SU] = U[0::NB, kc]
        u0a[:, i, :, SU:] = Ub[kc]
    for i, kc in enumerate(HCHUNKS):
        u0b[:, i, :, :SU] = U[0::NB, kc]
        u0b[:, i, :, SU : SU + 256] = Ub[kc]
        u0b[:, i, :, SU + 256 :] = Hb[i]
    u1 = np.ascontiguousarray(U[1::NB])
    return np.ascontiguousarray(u0a), np.ascontiguousarray(u0b), u1


def _build_nc() -> bass.Bass:
    nc = bacc.Bacc()
    u0a_in = nc.dram_tensor("u0a_in", [4, 128, SUA], DT_IN, kind="ExternalInput")
    u0b_in = nc.dram_tensor("u0b_in", [4, 128, SUB], DT_IN, kind="ExternalInput")
    u1_in = nc.dram_tensor("u1_in", [KC, 128, SU], DT_IN, kind="ExternalInput")
    scale_in = nc.dram_tensor("scale_in", [128, 2], F32, kind="ExternalInput")
    out = nc.dram_tensor("out", [NB, OUT_SEGS, HOP], F32, kind="ExternalOutput")

    with TileContext(nc) as tc:
        with (
            tc.tile_pool(name="up", bufs=1) as u_pool,
            tc.tile_pool(name="sp", bufs=1) as s_pool,
            tc.tile_pool(name="wu", bufs=1) as wu_pool,
            tc.tile_pool(name="ev", bufs=3) as ev_pool,
            tc.tile_pool(name="ps", bufs=8, space="PSUM") as psum_pool,
        ):
            # One DMA per chunk (8 in flight = all HWDGE sem lanes);
            # batch-0 tiles carry their own basis columns. Chunks
            # alternate between the SP and ACT HWDGE rings so triggers
            # (~650ns of engine issue time each) pipeline twice as fast.
            u_sb = [[None] * KC for _ in range(NB)]
            for kc in range(KC):
                w = SUA if kc in ACHUNKS else SUB
                ut = u_pool.tile([128, w], DT_IN, name=f"u0_{kc}",
                                 tag=f"u0_{kc}")
                src = (u0a_in[ACHUNKS.index(kc)] if kc in ACHUNKS
                       else u0b_in[HCHUNKS.index(kc)])
                eng = nc.sync if kc % 2 == 0 else nc.scalar
                eng.dma_start(ut[:, :], src)
                u_sb[0][kc] = ut
            # batch-1 chunks go via SWDGE behind a gate-copy that depends
            # on batch-0's last chunk: they must not steal HBM bandwidth
            # from batch-0's critical-path transfers.
            gate = wu_pool.tile([128, 2], DT_IN, name="gate", tag="gate")
            nc.gpsimd.tensor_copy(gate[:, :], u_sb[0][KC - 1][:, 0:2])
            for kc in range(KC):
                ut = u_pool.tile([128, SU], DT_IN, name=f"u1_{kc}",
                                 tag=f"u1_{kc}")
                nc.gpsimd.dma_start(ut[:, :], u1_in[kc])
                u_sb[1][kc] = ut

            def ub_ap(kc):
                return u_sb[0][kc][:, SU : SU + 256]

            def hb_ap(hi):
                return u_sb[0][HCHUNKS[hi]][:, SU + 256 : SU + 512]

            # scale via SWDGE (separate sem space, off the critical path)
            scale_sb = s_pool.tile([128, 2], F32, name="scale_sb", tag="scale_sb")
            scale_wu = s_pool.tile([128, 2], F32, name="scale_wu", tag="scale_wu")
            nc.gpsimd.dma_start(scale_sb[:, :], scale_in[:, :])
            # ACT warm-up read of the scale table so later edge-tile
            # activations on ScalarE don't each need the DMA-sem wait.
            nc.scalar.copy(scale_wu[:, :], scale_sb[:, :])

            # PE warm-up: ~3.4us of dummy matmuls on zeroed scratch while
            # the first u chunk is still in flight, so the HAM clock gate
            # reaches 8/8 (2.4 GHz) before the first real matmul.
            wu_w = wu_pool.tile([128, 128], DT_IN, name="wu_w", tag="wu_w")
            wu_r = wu_pool.tile([128, 256], DT_IN, name="wu_r", tag="wu_r")
            nc.vector.memset(wu_w[:, :], 0)
            nc.vector.memset(wu_r[:, :], 0)
            wu_ps = psum_pool.tile([128, HOP], F32, name="wu_ps", tag="psum")
            for i in range(16):
                nc.tensor.matmul(
                    wu_ps[:, :], wu_w[:, :], wu_r[:, :],
                    start=(i == 0), stop=(i == 15),
                )

            # `start=True` clears the whole PSUM bank, so each s-tile
            # owns a full bank: up to 8 concurrently-accumulating s-tiles
            # per group. Chunk-outer / s-tile-inner order matches DMA
            # arrival so matmuls start on chunk 0. The final group is
            # only 2 s-tiles so the last output write (the un-overlapped
            # tail) is small.
            batch_groups = [
                [list(range(0, 8)), list(range(8, 16))],
                [list(range(0, 8)), list(range(8, 14)), [14, 15]],
            ]
            for b in range(NB):
                for sts in batch_groups[b]:
                    psums = {
                        st: psum_pool.tile([128, HOP], F32,
                                           name=f"ps{b}_{st}", tag="psum")
                        for st in sts
                    }
                    # sweep order interleaves H sweeps (which only need
                    # already-arrived chunks 2,3) into the DMA-paced U
                    # sweep sequence so batch-0 group-A stalls get filled.
                    sweeps = [("U", 0), ("U", 1), ("U", 2), ("U", 3),
                              ("H", 0), ("H", 1), ("U", 4), ("U", 5),
                              ("U", 6), ("U", 7), ("H", 2), ("H", 3)]
                    for si, (kind, idx) in enumerate(sweeps):
                        for st in sts:
                            s0 = 2 + 128 * st
                            if kind == "U":
                                lhsT = u_sb[b][idx][:, s0 + 1 : s0 + 129]
                                rhs = ub_ap(idx)
                            else:
                                uc = HCHUNKS[idx]
                                lhsT = u_sb[b][uc][:, s0 : s0 + 128]
                                rhs = hb_ap(idx)
                            nc.tensor.matmul(
                                psums[st][:, :], lhsT, rhs,
                                start=(si == 0),
                                stop=(si == len(sweeps) - 1),
                            )
                    # evict into one wide staging tile: bases carry the
                    # steady-state 0.25; the two edge s-tiles get a
                    # per-partition fixup via ScalarE's activation scale
                    # vector. Plain copies alternate ScalarE/VectorE so
                    # the eviction burst drains at 2x rate. One batched
                    # out-DMA per group (vs 8) kills trigger serialization.
                    ng = len(sts)
                    ev = ev_pool.tile([128, ng * HOP], F32, name="ev",
                                      tag=f"ev{ng}")
                    for i, st in enumerate(sts):
                        evs = ev[:, i * HOP : (i + 1) * HOP]
                        if st == 0:
                            nc.scalar.mul(evs, psums[st][:, :], scale_sb[:, 0:1])
                        elif st == NT - 1:
                            nc.scalar.mul(evs, psums[st][:, :], scale_sb[:, 1:2])
                        elif st % 2 == 0:
                            nc.vector.tensor_copy(evs, psums[st][:, :])
                        else:
                            nc.scalar.copy(evs, psums[st][:, :])
                    nfull = sum(1 for st in sts if st != NT - 1)
                    seg0 = 128 * sts[0]
                    if nfull >= 2:
                        nc.sync.dma_start(
                            out[b, seg0 : seg0 + 128 * nfull, :].rearrange(
                                "(k p) r -> p k r", p=128),
                            ev[:, : nfull * HOP].rearrange(
                                "p (k r) -> p k r", k=nfull),
                        )
                    elif nfull == 1:
                        nc.sync.dma_start(
                            out[b, seg0 : seg0 + 128, :], ev[:, :HOP]
                        )
                    if sts[-1] == NT - 1:
                        # last s-tile has 81 valid segments; its small
                        # write rides the ACT ring, parallel to the big one
                        nc.scalar.dma_start(
                            out[b, 1920:2001, :],
                            ev[:81, (ng - 1) * HOP : ng * HOP],
                        )
    nc.finalize()
    return nc


def _run(inputs: dict, trace: bool = False):
    stft = np.asarray(inputs["stft_matrix"], dtype=np.float32)
    perm, Ub, Hb = _make_bases()
    U = _prep_u(stft, perm)
    u0a, u0b, u1 = _fuse_inputs(U, Ub, Hb)
    scales = _make_scales()
    in_maps = [
        {"u0a_in": u0a[c], "u0b_in": u0b[c], "u1_in": u1[c],
         "scale_in": scales}
        for c in range(NCORES)
    ]
    nc = _build_nc()
    res = bass_utils.run_bass_kernel_spmd(
        nc, in_maps, core_ids=list(range(NCORES)), trace=trace
    )
    out = np.concatenate(
        [res.results[c]["out"].reshape(NB, OUT_LEN) for c in range(NCORES)], axis=0
    )
    return out, res


def kernel(**inputs) -> np.ndarray:
    out, _ = _run(inputs, trace=False)
    return out


# revision 21
# speedup vs baseline: 1.0308x; 1.0308x over previous
"""Inverse STFT (nn_InverseSTFT) as a Bass/Tile kernel on 8 TRN2 NeuronCores.

Math
----
Reference computes, per batch b:
  full spectrum from one-sided stft via conjugate symmetry (F = 1024),
  ytmp[w, t] = sum_{f,c} full[f, t, c] * basis[f, w, c]          (IDFT)
  y = overlap_add(ytmp, hop=256), window-sum normalize, trim n_fft//2.

Folding the conjugate symmetry into the basis gives an exact K=1024 real
matmul; every folded-basis row k is a single-frequency sinusoid
g_k[w] = gamma * cos/sin(2*pi*f_k*w/1024). With hop = 1024/4, writing
w = 256*j + r factors each row as
  g_k[256 j + r] = cos(pi f_k j / 2) * g_k[r] + sin(pi f_k j / 2) * h_k[r]
with coefficients in {-1, 0, 1} determined by f_k mod 4 (h_k is the
quadrature partner of g_k). The overlap-add over j therefore collapses
into a shifted-add prefilter on the frames (computed on HOST, since it is
a cheap linear repack of the input) followed by matmuls of only
  K=1024 (U part)  +  K=512 (H part; only odd f has sin coefficients)
per 256-wide output segment, instead of 4 * K=1024:
  u[k, s] = sum_j c_{kj} x[k, s-j]   (c patterns by f mod 4:
            [1,1,1,1] / [1,0,-1,0] / [1,-1,1,-1] / [1,0,-1,0])
  v[k, s] = sum_j s_{kj} x[k, s-j] = u[k, s-1]  for odd f (free shift!)
  y[256 s + r] = sum_k Ub[k, r] u[k, s] + sum_{odd f} Hb[k, r] u[k, s-1]
K rows are permuted so each 128-chunk holds a single f-mod-4 class
(classes have exactly 256 rows each); the sign for f==3 mod 4's v is
folded into Hb. This is 12 accumulating chunk-matmuls per psum tile
instead of 32 -> 2.67x fewer TensorE cycles.

Schedule: chunk-outer / s-tile-inner with all 16 psum tiles (= all 8
PSUM banks) live per batch, so matmuls start as soon as u-chunk 0 lands
instead of waiting for the whole batch's DMA.

Window-sum normalization = 0.25 folded into the bases; per-partition
fixup on the two edge s-tiles. Output keeps segments s = 2..2002.

Sharding: pure data parallel, 2 batches per core.
"""

import numpy as np

import concourse.bass as bass
import concourse.mybir as mybir
from concourse.tile import TileContext
from concourse import bacc, bass_utils

N_FFT = 1024
HOP = 256
B = 16
NFREQ = 513
T = 2000
NCORES = 8
NB = B // NCORES          # batches per core
KC = 8                    # K chunks of 128 (K = 1024)
SU = 2052                 # u free size: i in [0, 2052), i <-> s = i - 1
SEG = 2003                # total segments in un-trimmed output
OUT_SEGS = 2001           # segments s = 2..2002
NT = 16                   # s-tiles of 128 per batch (last has 81 valid rows)
OUT_LEN = OUT_SEGS * HOP  # 512256
HCHUNKS = (2, 3, 6, 7)    # u chunks (f mod 4 == 1 or 3) used by the H part

F32 = mybir.dt.float32
DT_IN = mybir.dt.bfloat16

import ml_dtypes

NP_IN = ml_dtypes.bfloat16


def _make_bases():
    """(8,128,256) U basis and (4,128,256) H basis, 0.25 wss folded in.

    Row k of the folded basis (k<=512: cos rows f=k; k>512: sin rows
    f=k-512) restricted to r in [0,256), plus its quadrature partner.
    Rows permuted so chunks 0-1 = f%4==0, 2-3 = f%4==1, 4-5 = f%4==2,
    6-7 = f%4==3; the f%4==3 sin-coefficient sign is folded into Hb.
    """
    fk = np.concatenate([np.arange(513), np.arange(1, 512)])
    is_sin = np.concatenate([np.zeros(513, bool), np.ones(511, bool)])
    k = np.arange(1024)
    gamma = np.where((k == 0) | (k == 512), 1.0 / 1024, 2.0 / 1024)
    gamma = np.where(is_sin, -2.0 / 1024, gamma)
    r = np.arange(256)
    th = 2 * np.pi * np.outer(fk, r) / 1024.0
    g = np.where(is_sin[:, None], gamma[:, None] * np.sin(th),
                 gamma[:, None] * np.cos(th))
    h = np.where(is_sin[:, None], gamma[:, None] * np.cos(th),
                 -gamma[:, None] * np.sin(th))
    cls = fk % 4
    perm = np.concatenate([np.where(cls == c)[0] for c in range(4)])
    gp, hp, clsp = g[perm], h[perm], cls[perm]
    Ub = (gp * 0.25).reshape(KC, 128, 256)
    hrows = np.concatenate([np.where(clsp == 1)[0], np.where(clsp == 3)[0]])
    sign = np.where(clsp[hrows] == 1, 1.0, -1.0)[:, None]
    Hb = (hp[hrows] * sign * 0.25).reshape(4, 128, 256)
    return perm, Ub.astype(NP_IN), Hb.astype(NP_IN)


def _make_scales() -> np.ndarray:
    """(128, 2) per-partition wss fixup (on top of the 0.25 in the bases).

    col 0 -> first s-tile (s = 2..129): s=2 has 3 frames -> 4/3.
    col 1 -> last s-tile (s = 1922..2002): s=2000 -> 4/3, 2001 -> 2, 2002 -> 4.
    """
    sc = np.ones((128, 2), np.float32)
    sc[0, 0] = np.float32(4.0) / np.float32(3.0)
    sc[78, 1] = np.float32(4.0) / np.float32(3.0)
    sc[79, 1] = 2.0
    sc[80, 1] = 4.0
    return sc


def _prep_u(stft: np.ndarray, perm: np.ndarray) -> np.ndarray:
    """(16,513,2000,2) f32 -> (16, KC, 128, SU) prefiltered u, bf16.

    u[k, i] <-> u[k, s = i-1] = sum_j c_{kj} x[k, s-j], x zero outside
    [0, T). Computed in f32, cast to bf16 at the end.
    """
    re = stft[:, :, :, 0]                  # (B, 513, T)
    im = stft[:, 1:512, :, 1]              # (B, 511, T)
    xk = np.concatenate([re, im], axis=1)  # (B, 1024, T)
    xp = np.zeros((B, 1024, 2056), np.float32)
    xp[:, :, 4 : 4 + T] = xk[:, perm, :]   # xp[:, :, t+4] = x[t]
    u = np.empty((B, 1024, SU), np.float32)
    x0 = xp[:, :, 3 : 3 + SU]              # x[s]
    x1 = xp[:, :, 2 : 2 + SU]              # x[s-1]
    x2 = xp[:, :, 1 : 1 + SU]              # x[s-2]
    x3 = xp[:, :, 0 : SU]                  # x[s-3]
    u[:, 0:256] = x0[:, 0:256] + x1[:, 0:256] + x2[:, 0:256] + x3[:, 0:256]
    u[:, 512:768] = (x0[:, 512:768] - x1[:, 512:768]
                     + x2[:, 512:768] - x3[:, 512:768])
    u[:, 256:512] = x0[:, 256:512] - x2[:, 256:512]
    u[:, 768:1024] = x0[:, 768:1024] - x2[:, 768:1024]
    return u.reshape(B, KC, 128, SU).astype(NP_IN)


ACHUNKS = (0, 1, 4, 5)    # u chunks whose batch-0 tile carries u + Ub
SUA = SU + 256
SUB = SU + 512


def _fuse_inputs(U: np.ndarray, Ub: np.ndarray, Hb: np.ndarray):
    """Per-core input arrays: batch-0 chunk tiles carry their own basis
    columns (Ub, and Hb for the odd-class chunks) so one DMA per chunk
    delivers everything that chunk's matmul sweeps need."""
    ncores_in = U.shape[0] // NB
    u0a = np.zeros((ncores_in, 4, 128, SUA), NP_IN)
    u0b = np.zeros((ncores_in, 4, 128, SUB), NP_IN)
    for i, kc in enumerate(ACHUNKS):
        u0a[:, i, :, :SU] = U[0::NB, kc]
        u0a[:, i, :, SU:] = Ub[kc]
    for i, kc in enumerate(HCHUNKS):
        u0b[:, i, :, :SU] = U[0::NB, kc]
        u0b[:, i, :, SU : SU + 256] = Ub[kc]
        u0b[:, i, :, SU + 256 :] = Hb[i]
    u1 = U[1::NB].transpose(0, 2, 1, 3).reshape(ncores_in, 128, KC * SU)
    return np.ascontiguousarray(u0a), np.ascontiguousarray(u0b), np.ascontiguousarray(u1)


def _build_nc() -> bass.Bass:
    nc = bacc.Bacc()
    u0a_in = nc.dram_tensor("u0a_in", [4, 128, SUA], DT_IN, kind="ExternalInput")
    u0b_in = nc.dram_tensor("u0b_in", [4, 128, SUB], DT_IN, kind="ExternalInput")
    u1_in = nc.dram_tensor("u1_in", [128, KC * SU], DT_IN, kind="ExternalInput")
    scale_in = nc.dram_tensor("scale_in", [128, 2], F32, kind="ExternalInput")
    out = nc.dram_tensor("out", [NB, OUT_SEGS, HOP], F32, kind="ExternalOutput")

    with TileContext(nc) as tc:
        with (
            tc.tile_pool(name="up", bufs=1) as u_pool,
            tc.tile_pool(name="sp", bufs=1) as s_pool,
            tc.tile_pool(name="wu", bufs=1) as wu_pool,
            tc.tile_pool(name="ev", bufs=3) as ev_pool,
            tc.tile_pool(name="ps", bufs=8, space="PSUM") as psum_pool,
        ):
            # One DMA per chunk (8 in flight = all HWDGE sem lanes);
            # batch-0 tiles carry their own basis columns. Chunks
            # alternate between the SP and ACT HWDGE rings so triggers
            # (~650ns of engine issue time each) pipeline twice as fast.
            u_sb = [[None] * KC for _ in range(NB)]
            for kc in range(KC):
                w = SUA if kc in ACHUNKS else SUB
                ut = u_pool.tile([128, w], DT_IN, name=f"u0_{kc}",
                                 tag=f"u0_{kc}")
                src = (u0a_in[ACHUNKS.index(kc)] if kc in ACHUNKS
                       else u0b_in[HCHUNKS.index(kc)])
                eng = nc.sync if kc % 2 == 0 else nc.scalar
                eng.dma_start(ut[:, :], src)
                u_sb[0][kc] = ut
            # batch-1 arrives as ONE wide DMA: it occupies a single HWDGE
            # queue, so packet round-robin gives it only a small share of
            # HBM bandwidth while batch-0's critical-path chunks drain,
            # yet it completes long before batch-1 compute starts.
            u1_all = u_pool.tile([128, KC * SU], DT_IN, name="u1", tag="u1")
            nc.sync.dma_start(u1_all[:, :], u1_in[:, :])
            for kc in range(KC):
                u_sb[1][kc] = u1_all[:, kc * SU : (kc + 1) * SU]

            def ub_ap(kc):
                return u_sb[0][kc][:, SU : SU + 256]

            def hb_ap(hi):
                return u_sb[0][HCHUNKS[hi]][:, SU + 256 : SU + 512]

            # scale via SWDGE (separate sem space, off the critical path)
            scale_sb = s_pool.tile([128, 2], F32, name="scale_sb", tag="scale_sb")
            scale_wu = s_pool.tile([128, 2], F32, name="scale_wu", tag="scale_wu")
            nc.gpsimd.dma_start(scale_sb[:, :], scale_in[:, :])
            # ACT warm-up read of the scale table so later edge-tile
            # activations on ScalarE don't each need the DMA-sem wait.
            nc.scalar.copy(scale_wu[:, :], scale_sb[:, :])

            # PE warm-up: ~3.4us of dummy matmuls on zeroed scratch while
            # the first u chunk is still in flight, so the HAM clock gate
            # reaches 8/8 (2.4 GHz) before the first real matmul.
            wu_w = wu_pool.tile([128, 128], DT_IN, name="wu_w", tag="wu_w")
            wu_r = wu_pool.tile([128, 256], DT_IN, name="wu_r", tag="wu_r")
            nc.vector.memset(wu_w[:, :], 0)
            nc.vector.memset(wu_r[:, :], 0)
            wu_ps = psum_pool.tile([128, HOP], F32, name="wu_ps", tag="psum")
            for i in range(16):
                nc.tensor.matmul(
                    wu_ps[:, :], wu_w[:, :], wu_r[:, :],
                    start=(i == 0), stop=(i == 15),
                )

            # `start=True` clears the whole PSUM bank, so each s-tile
            # owns a full bank: up to 8 concurrently-accumulating s-tiles
            # per group. Chunk-outer / s-tile-inner order matches DMA
            # arrival so matmuls start on chunk 0. The final group is
            # only 2 s-tiles so the last output write (the un-overlapped
            # tail) is small.
            batch_groups = [
                [list(range(0, 8)), list(range(8, 16))],
                [list(range(0, 8)), list(range(8, 14)), [14, 15]],
            ]
            for b in range(NB):
                for sts in batch_groups[b]:
                    psums = {
                        st: psum_pool.tile([128, HOP], F32,
                                           name=f"ps{b}_{st}", tag="psum")
                        for st in sts
                    }
                    # sweep order interleaves H sweeps (which only need
                    # already-arrived chunks 2,3) into the DMA-paced U
                    # sweep sequence so batch-0 group-A stalls get filled.
                    sweeps = [("U", 0), ("U", 1), ("U", 2), ("U", 3),
                              ("H", 0), ("H", 1), ("U", 4), ("U", 5),
                              ("U", 6), ("U", 7), ("H", 2), ("H", 3)]
                    for si, (kind, idx) in enumerate(sweeps):
                        for st in sts:
                            s0 = 2 + 128 * st
                            if kind == "U":
                                lhsT = u_sb[b][idx][:, s0 + 1 : s0 + 129]
                                rhs = ub_ap(idx)
                            else:
                                uc = HCHUNKS[idx]
                                lhsT = u_sb[b][uc][:, s0 : s0 + 128]
                                rhs = hb_ap(idx)
                            nc.tensor.matmul(
                                psums[st][:, :], lhsT, rhs,
                                start=(si == 0),
                                stop=(si == len(sweeps) - 1),
                            )
                    # evict into one wide staging tile: bases carry the
                    # steady-state 0.25; the two edge s-tiles get a
                    # per-partition fixup via ScalarE's activation scale
                    # vector. Plain copies alternate ScalarE/VectorE so
                    # the eviction burst drains at 2x rate. One batched
                    # out-DMA per group (vs 8) kills trigger serialization.
                    ng = len(sts)
                    ev = ev_pool.tile([128, ng * HOP], F32, name="ev",
                                      tag=f"ev{ng}")
                    for i, st in enumerate(sts):
                        evs = ev[:, i * HOP : (i + 1) * HOP]
                        if st == 0:
                            nc.scalar.mul(evs, psums[st][:, :], scale_sb[:, 0:1])
                        elif st == NT - 1:
                            nc.scalar.mul(evs, psums[st][:, :], scale_sb[:, 1:2])
                        elif st % 2 == 0:
                            nc.vector.tensor_copy(evs, psums[st][:, :])
                        else:
                            nc.scalar.copy(evs, psums[st][:, :])
                    nfull = sum(1 for st in sts if st != NT - 1)
                    seg0 = 128 * sts[0]
                    if nfull >= 2:
                        nc.sync.dma_start(
                            out[b, seg0 : seg0 + 128 * nfull, :].rearrange(
                                "(k p) r -> p k r", p=128),
                            ev[:, : nfull * HOP].rearrange(
                                "p (k r) -> p k r", k=nfull),
                        )
                    elif nfull == 1:
                        nc.sync.dma_start(
                            out[b, seg0 : seg0 + 128, :], ev[:, :HOP]
                        )
                    if sts[-1] == NT - 1:
                        # last s-tile has 81 valid segments; its small
                        # write rides the ACT ring, parallel to the big one
                        nc.scalar.dma_start(
                            out[b, 1920:2001, :],
                            ev[:81, (ng - 1) * HOP : ng * HOP],
                        )
    nc.finalize()
    return nc


def _run(inputs: dict, trace: bool = False):
    stft = np.asarray(inputs["stft_matrix"], dtype=np.float32)
    perm, Ub, Hb = _make_bases()
    U = _prep_u(stft, perm)
    u0a, u0b, u1 = _fuse_inputs(U, Ub, Hb)
    scales = _make_scales()
    in_maps = [
        {"u0a_in": u0a[c], "u0b_in": u0b[c], "u1_in": u1[c],
         "scale_in": scales}
        for c in range(NCORES)
    ]
    nc = _build_nc()
    res = bass_utils.run_bass_kernel_spmd(
        nc, in_maps, core_ids=list(range(NCORES)), trace=trace
    )
    out = np.concatenate(
        [res.results[c]["out"].reshape(NB, OUT_LEN) for c in range(NCORES)], axis=0
    )
    return out, res


def kernel(**inputs) -> np.ndarray:
    out, _ = _run(inputs, trace=False)
    return out


# revision 23
# speedup vs baseline: 1.2263x; 1.1897x over previous
"""Inverse STFT (nn_InverseSTFT) as a Bass/Tile kernel on 8 TRN2 NeuronCores.

Math
----
Reference computes, per batch b:
  full spectrum from one-sided stft via conjugate symmetry (F = 1024),
  ytmp[w, t] = sum_{f,c} full[f, t, c] * basis[f, w, c]          (IDFT)
  y = overlap_add(ytmp, hop=256), window-sum normalize, trim n_fft//2.

Folding the conjugate symmetry into the basis gives an exact K=1024 real
matmul; every folded-basis row k is a single-frequency sinusoid
g_k[w] = gamma * cos/sin(2*pi*f_k*w/1024). With hop = 1024/4, writing
w = 256*j + r factors each row as
  g_k[256 j + r] = cos(pi f_k j / 2) * g_k[r] + sin(pi f_k j / 2) * h_k[r]
with coefficients in {-1, 0, 1} determined by f_k mod 4 (h_k is the
quadrature partner of g_k). The overlap-add over j therefore collapses
into a shifted-add prefilter on the frames (computed on HOST, since it is
a cheap linear repack of the input) followed by matmuls of only
  K=1024 (U part)  +  K=512 (H part; only odd f has sin coefficients)
per 256-wide output segment, instead of 4 * K=1024:
  u[k, s] = sum_j c_{kj} x[k, s-j]   (c patterns by f mod 4:
            [1,1,1,1] / [1,0,-1,0] / [1,-1,1,-1] / [1,0,-1,0])
  v[k, s] = sum_j s_{kj} x[k, s-j] = u[k, s-1]  for odd f (free shift!)
  y[256 s + r] = sum_k Ub[k, r] u[k, s] + sum_{odd f} Hb[k, r] u[k, s-1]
K rows are permuted so each 128-chunk holds a single f-mod-4 class
(classes have exactly 256 rows each); the sign for f==3 mod 4's v is
folded into Hb. This is 12 accumulating chunk-matmuls per psum tile
instead of 32 -> 2.67x fewer TensorE cycles.

Schedule: chunk-outer / s-tile-inner with all 16 psum tiles (= all 8
PSUM banks) live per batch, so matmuls start as soon as u-chunk 0 lands
instead of waiting for the whole batch's DMA.

Window-sum normalization = 0.25 folded into the bases; per-partition
fixup on the two edge s-tiles. Output keeps segments s = 2..2002.

Sharding: pure data parallel, 2 batches per core.
"""

import numpy as np

import concourse.bass as bass
import concourse.mybir as mybir
from concourse.tile import TileContext
from concourse import bacc, bass_utils

N_FFT = 1024
HOP = 256
B = 16
NFREQ = 513
T = 2000
NCORES = 8
NB = B // NCORES          # batches per core
KC = 8                    # K chunks of 128 (K = 1024)
SU = 2052                 # u free size: i in [0, 2052), i <-> s = i - 1
SEG = 2003                # total segments in un-trimmed output
OUT_SEGS = 2001           # segments s = 2..2002
NT = 16                   # s-tiles of 128 per batch (last has 81 valid rows)
OUT_LEN = OUT_SEGS * HOP  # 512256
HCHUNKS = (2, 3, 6, 7)    # u chunks (f mod 4 == 1 or 3) used by the H part

F32 = mybir.dt.float32
DT_IN = mybir.dt.bfloat16

import ml_dtypes

NP_IN = ml_dtypes.bfloat16


def _make_bases():
    """(8,128,256) U basis and (4,128,256) H basis, 0.25 wss folded in.

    Row k of the folded basis (k<=512: cos rows f=k; k>512: sin rows
    f=k-512) restricted to r in [0,256), plus its quadrature partner.
    Rows permuted so chunks 0-1 = f%4==0, 2-3 = f%4==1, 4-5 = f%4==2,
    6-7 = f%4==3; the f%4==3 sin-coefficient sign is folded into Hb.
    """
    fk = np.concatenate([np.arange(513), np.arange(1, 512)])
    is_sin = np.concatenate([np.zeros(513, bool), np.ones(511, bool)])
    k = np.arange(1024)
    gamma = np.where((k == 0) | (k == 512), 1.0 / 1024, 2.0 / 1024)
    gamma = np.where(is_sin, -2.0 / 1024, gamma)
    r = np.arange(256)
    th = 2 * np.pi * np.outer(fk, r) / 1024.0
    g = np.where(is_sin[:, None], gamma[:, None] * np.sin(th),
                 gamma[:, None] * np.cos(th))
    h = np.where(is_sin[:, None], gamma[:, None] * np.cos(th),
                 -gamma[:, None] * np.sin(th))
    cls = fk % 4
    perm = np.concatenate([np.where(cls == c)[0] for c in range(4)])
    gp, hp, clsp = g[perm], h[perm], cls[perm]
    Ub = (gp * 0.25).reshape(KC, 128, 256)
    hrows = np.concatenate([np.where(clsp == 1)[0], np.where(clsp == 3)[0]])
    sign = np.where(clsp[hrows] == 1, 1.0, -1.0)[:, None]
    Hb = (hp[hrows] * sign * 0.25).reshape(4, 128, 256)
    return perm, Ub.astype(NP_IN), Hb.astype(NP_IN)


def _make_scales() -> np.ndarray:
    """(128, 2) per-partition wss fixup (on top of the 0.25 in the bases).

    col 0 -> first s-tile (s = 2..129): s=2 has 3 frames -> 4/3.
    col 1 -> last s-tile (s = 1922..2002): s=2000 -> 4/3, 2001 -> 2, 2002 -> 4.
    """
    sc = np.ones((128, 2), np.float32)
    sc[0, 0] = np.float32(4.0) / np.float32(3.0)
    sc[78, 1] = np.float32(4.0) / np.float32(3.0)
    sc[79, 1] = 2.0
    sc[80, 1] = 4.0
    return sc


def _prep_u(stft: np.ndarray, perm: np.ndarray) -> np.ndarray:
    """(16,513,2000,2) f32 -> (16, KC, 128, SU) prefiltered u, bf16.

    u[k, i] <-> u[k, s = i-1] = sum_j c_{kj} x[k, s-j], x zero outside
    [0, T). Computed in f32, cast to bf16 at the end.
    """
    re = stft[:, :, :, 0]                  # (B, 513, T)
    im = stft[:, 1:512, :, 1]              # (B, 511, T)
    xk = np.concatenate([re, im], axis=1)  # (B, 1024, T)
    xp = np.zeros((B, 1024, 2056), np.float32)
    xp[:, :, 4 : 4 + T] = xk[:, perm, :]   # xp[:, :, t+4] = x[t]
    u = np.empty((B, 1024, SU), np.float32)
    x0 = xp[:, :, 3 : 3 + SU]              # x[s]
    x1 = xp[:, :, 2 : 2 + SU]              # x[s-1]
    x2 = xp[:, :, 1 : 1 + SU]              # x[s-2]
    x3 = xp[:, :, 0 : SU]                  # x[s-3]
    u[:, 0:256] = x0[:, 0:256] + x1[:, 0:256] + x2[:, 0:256] + x3[:, 0:256]
    u[:, 512:768] = (x0[:, 512:768] - x1[:, 512:768]
                     + x2[:, 512:768] - x3[:, 512:768])
    u[:, 256:512] = x0[:, 256:512] - x2[:, 256:512]
    u[:, 768:1024] = x0[:, 768:1024] - x2[:, 768:1024]
    return u.reshape(B, KC, 128, SU).astype(NP_IN)


ACHUNKS = (0, 1, 4, 5)    # u chunks whose batch-0 tile carries u + Ub
SUA = SU + 256
SUB = SU + 512


def _fuse_inputs(U: np.ndarray, Ub: np.ndarray, Hb: np.ndarray):
    """Per-core input arrays: batch-0 chunk tiles carry their own basis
    columns (Ub, and Hb for the odd-class chunks) so one DMA per chunk
    delivers everything that chunk's matmul sweeps need."""
    ncores_in = U.shape[0] // NB
    u0a = np.zeros((ncores_in, 4, 128, SUA), NP_IN)
    u0b = np.zeros((ncores_in, 4, 128, SUB), NP_IN)
    for i, kc in enumerate(ACHUNKS):
        u0a[:, i, :, :SU] = U[0::NB, kc]
        u0a[:, i, :, SU:] = Ub[kc]
    for i, kc in enumerate(HCHUNKS):
        u0b[:, i, :, :SU] = U[0::NB, kc]
        u0b[:, i, :, SU : SU + 256] = Ub[kc]
        u0b[:, i, :, SU + 256 :] = Hb[i]
    u1 = U[1::NB].transpose(0, 2, 1, 3).reshape(ncores_in, 128, KC * SU)
    return np.ascontiguousarray(u0a), np.ascontiguousarray(u0b), np.ascontiguousarray(u1)


def _build_nc() -> bass.Bass:
    nc = bacc.Bacc()
    u0a_in = nc.dram_tensor("u0a_in", [4, 128, SUA], DT_IN, kind="ExternalInput")
    u0b_in = nc.dram_tensor("u0b_in", [4, 128, SUB], DT_IN, kind="ExternalInput")
    u1_in = nc.dram_tensor("u1_in", [128, KC * SU], DT_IN, kind="ExternalInput")
    scale_in = nc.dram_tensor("scale_in", [128, 2], F32, kind="ExternalInput")
    out = nc.dram_tensor("out", [NB, OUT_SEGS, HOP], F32, kind="ExternalOutput")

    with TileContext(nc) as tc:
        with (
            tc.tile_pool(name="up", bufs=1) as u_pool,
            tc.tile_pool(name="sp", bufs=1) as s_pool,
            tc.tile_pool(name="wu", bufs=1) as wu_pool,
            tc.tile_pool(name="ev", bufs=3) as ev_pool,
            tc.tile_pool(name="ps", bufs=8, space="PSUM") as psum_pool,
        ):
            # One DMA per chunk (8 in flight = all HWDGE sem lanes);
            # batch-0 tiles carry their own basis columns. Chunks
            # alternate between the SP and ACT HWDGE rings so triggers
            # (~650ns of engine issue time each) pipeline twice as fast.
            u_sb = [[None] * KC for _ in range(NB)]
            for kc in range(KC):
                w = SUA if kc in ACHUNKS else SUB
                ut = u_pool.tile([128, w], DT_IN, name=f"u0_{kc}",
                                 tag=f"u0_{kc}")
                src = (u0a_in[ACHUNKS.index(kc)] if kc in ACHUNKS
                       else u0b_in[HCHUNKS.index(kc)])
                eng = nc.sync if kc % 2 == 0 else nc.scalar
                eng.dma_start(ut[:, :], src)
                u_sb[0][kc] = ut
            # batch-1 arrives as ONE wide DMA: it occupies a single HWDGE
            # queue, so packet round-robin gives it only a small share of
            # HBM bandwidth while batch-0's critical-path chunks drain,
            # yet it completes long before batch-1 compute starts.
            u1_all = u_pool.tile([128, KC * SU], DT_IN, name="u1", tag="u1")
            nc.sync.dma_start(u1_all[:, :], u1_in[:, :])
            for kc in range(KC):
                u_sb[1][kc] = u1_all[:, kc * SU : (kc + 1) * SU]

            def ub_ap(kc):
                return u_sb[0][kc][:, SU : SU + 256]

            def hb_ap(hi):
                return u_sb[0][HCHUNKS[hi]][:, SU + 256 : SU + 512]

            # scale via SWDGE (separate sem space, off the critical path)
            scale_sb = s_pool.tile([128, 2], F32, name="scale_sb", tag="scale_sb")
            scale_wu = s_pool.tile([128, 2], F32, name="scale_wu", tag="scale_wu")
            nc.gpsimd.dma_start(scale_sb[:, :], scale_in[:, :])
            # ACT warm-up read of the scale table so later edge-tile
            # activations on ScalarE don't each need the DMA-sem wait.
            nc.scalar.copy(scale_wu[:, :], scale_sb[:, :])

            # PE warm-up: ~3.4us of dummy matmuls on zeroed scratch while
            # the first u chunk is still in flight, so the HAM clock gate
            # reaches 8/8 (2.4 GHz) before the first real matmul.
            wu_w = wu_pool.tile([128, 128], DT_IN, name="wu_w", tag="wu_w")
            wu_r = wu_pool.tile([128, 256], DT_IN, name="wu_r", tag="wu_r")
            nc.vector.memset(wu_w[:, :], 0)
            nc.vector.memset(wu_r[:, :], 0)
            wu_ps = psum_pool.tile([128, HOP], F32, name="wu_ps", tag="psum")
            for i in range(16):
                nc.tensor.matmul(
                    wu_ps[:, :], wu_w[:, :], wu_r[:, :],
                    start=(i == 0), stop=(i == 15),
                )

            # `start=True` clears the whole PSUM bank, so each s-tile
            # owns a full bank: up to 8 concurrently-accumulating s-tiles
            # per group. Chunk-outer / s-tile-inner order matches DMA
            # arrival so matmuls start on chunk 0. The final group is
            # only 2 s-tiles so the last output write (the un-overlapped
            # tail) is small.
            batch_groups = [
                [list(range(0, 8)), list(range(8, 16))],
                [list(range(0, 8)), list(range(8, 14)), [14, 15]],
            ]
            for b in range(NB):
                for sts in batch_groups[b]:
                    # two s-tiles share a PSUM bank: the even slot's first
                    # matmul uses start=True (clears the whole bank); the
                    # odd slot's chain runs all-start=False and relies on
                    # has_written=0 -> overwrite for its first matmul.
                    banks = {}
                    psums = {}
                    for i, st in enumerate(sts):
                        if i % 2 == 0:
                            banks[i // 2] = psum_pool.tile(
                                [128, 2 * HOP], F32,
                                name=f"ps{b}_{st}", tag="psum")
                            psums[st] = banks[i // 2][:, :HOP]
                        else:
                            psums[st] = banks[i // 2][:, HOP:]
                    first_sts = {sts[i] for i in range(0, len(sts), 2)}
                    # sweep order interleaves H sweeps (which only need
                    # already-arrived chunks 2,3) into the DMA-paced U
                    # sweep sequence so batch-0 group-A stalls get filled.
                    sweeps = [("U", 0), ("U", 1), ("U", 2), ("U", 3),
                              ("H", 0), ("H", 1), ("U", 4), ("U", 5),
                              ("U", 6), ("U", 7), ("H", 2), ("H", 3)]
                    for si, (kind, idx) in enumerate(sweeps):
                        for st in sts:
                            s0 = 2 + 128 * st
                            if kind == "U":
                                lhsT = u_sb[b][idx][:, s0 + 1 : s0 + 129]
                                rhs = ub_ap(idx)
                            else:
                                uc = HCHUNKS[idx]
                                lhsT = u_sb[b][uc][:, s0 : s0 + 128]
                                rhs = hb_ap(idx)
                            nc.tensor.matmul(
                                psums[st][:, :], lhsT, rhs,
                                start=(si == 0 and st in first_sts),
                                stop=(si == len(sweeps) - 1),
                                skip_group_check=True,
                            )
                    # evict into one wide staging tile: bases carry the
                    # steady-state 0.25; the two edge s-tiles get a
                    # per-partition fixup via ScalarE's activation scale
                    # vector. Plain copies alternate ScalarE/VectorE so
                    # the eviction burst drains at 2x rate. One batched
                    # out-DMA per group (vs 8) kills trigger serialization.
                    ng = len(sts)
                    ev = ev_pool.tile([128, ng * HOP], F32, name="ev",
                                      tag=f"ev{ng}")
                    for i, st in enumerate(sts):
                        evs = ev[:, i * HOP : (i + 1) * HOP]
                        if st == 0:
                            nc.scalar.mul(evs, psums[st][:, :], scale_sb[:, 0:1])
                        elif st == NT - 1:
                            nc.scalar.mul(evs, psums[st][:, :], scale_sb[:, 1:2])
                        elif st % 2 == 0:
                            nc.vector.tensor_copy(evs, psums[st][:, :])
                        else:
                            nc.scalar.copy(evs, psums[st][:, :])
                    nfull = sum(1 for st in sts if st != NT - 1)
                    seg0 = 128 * sts[0]
                    if nfull >= 2:
                        nc.sync.dma_start(
                            out[b, seg0 : seg0 + 128 * nfull, :].rearrange(
                                "(k p) r -> p k r", p=128),
                            ev[:, : nfull * HOP].rearrange(
                                "p (k r) -> p k r", k=nfull),
                        )
                    elif nfull == 1:
                        nc.sync.dma_start(
                            out[b, seg0 : seg0 + 128, :], ev[:, :HOP]
                        )
                    if sts[-1] == NT - 1:
                        # last s-tile has 81 valid segments; its small
                        # write rides the ACT ring, parallel to the big one
                        nc.scalar.dma_start(
                            out[b, 1920:2001, :],
                            ev[:81, (ng - 1) * HOP : ng * HOP],
                        )
    nc.finalize()
    return nc


def _run(inputs: dict, trace: bool = False):
    stft = np.asarray(inputs["stft_matrix"], dtype=np.float32)
    perm, Ub, Hb = _make_bases()
    U = _prep_u(stft, perm)
    u0a, u0b, u1 = _fuse_inputs(U, Ub, Hb)
    scales = _make_scales()
    in_maps = [
        {"u0a_in": u0a[c], "u0b_in": u0b[c], "u1_in": u1[c],
         "scale_in": scales}
        for c in range(NCORES)
    ]
    nc = _build_nc()
    res = bass_utils.run_bass_kernel_spmd(
        nc, in_maps, core_ids=list(range(NCORES)), trace=trace
    )
    out = np.concatenate(
        [res.results[c]["out"].reshape(NB, OUT_LEN) for c in range(NCORES)], axis=0
    )
    return out, res


def kernel(**inputs) -> np.ndarray:
    out, _ = _run(inputs, trace=False)
    return out
